# revision 1
# baseline (speedup 1.0000x reference)
"""Trainium2 Bass kernel for nn_MultiHeadAttention (B=2, T=2048, D=1024, H=16).

Sharding: 8 cores; core c owns head pair (2c, 2c+1) = output-channel slice
[c*128, (c+1)*128) of Wq/Wk/Wv columns and Wo rows (tensor parallel), both
batches. Host pre-transposes x and weight slices; each core computes a
partial output projection over its 128 ctx channels; host sums the 8
partials (replaces the all-reduce) and adds bo.

Per-core dataflow (all matmuls float32r, moving N=512):
  QT/KT[e,t] projections (xT moving), VT projection + PE-transpose to V
  natural [t,e] with a fused ones-column for the softmax denominator;
  per (batch, 1024-wide q-pair): scoresT[k,q] = KT.T @ QT row-tiled 2 heads
  into 2-bank PSUM, exp on ACT over [128,1024] (scale=1/8 fused), ctx
  accumulation ctxU_aug[65,1024] = [V|1].T @ escT over 16 k-tiles; 1/s via
  DVE reciprocal + PE outer-product broadcast; out-proj partial [t,e] =
  ctxT.T @ WoT_slice streamed to DRAM.
"""

import numpy as np

P = 128
D = 1024
BT = 4096
T = 2048
NB = 2
DC = 8    # D chunks of 128
TCH = 8   # 512-wide t-chunks over BT
KT = 16   # 128-wide k-tiles per batch
QC = 4    # 512-wide q-chunks per batch
NCORES = 8
DK = 64

_CACHE = {}


def _build(reps=1):
    import concourse.bass as bass
    import concourse.tile as tile
    from concourse import bacc, mybir
    from concourse.masks import make_identity

    f32 = mybir.dt.float32
    f32r = mybir.dt.float32r
    f16 = mybir.dt.float16
    Exp = mybir.ActivationFunctionType.Exp
    ds = bass.ds

    nc = bacc.Bacc("TRN2", target_bir_lowering=False, debug=False)

    xt = nc.dram_tensor("xt", [D, BT], f32r, kind="ExternalInput").ap()
    wq = nc.dram_tensor("wq", [D, P], f32r, kind="ExternalInput").ap()
    wk = nc.dram_tensor("wk", [D, P], f32r, kind="ExternalInput").ap()
    wv = nc.dram_tensor("wv", [D, P], f32r, kind="ExternalInput").ap()
    wo = nc.dram_tensor("wo", [P, D], f32r, kind="ExternalInput").ap()
    bqd = nc.dram_tensor("bq", [P, 1], f32, kind="ExternalInput").ap()
    bkd = nc.dram_tensor("bk", [P, 1], f32, kind="ExternalInput").ap()
    bvd = nc.dram_tensor("bv", [P, 1], f32, kind="ExternalInput").ap()
    out = nc.dram_tensor("out", [BT, D], f32, kind="ExternalOutput").ap()

    with tile.TileContext(nc) as tc:
        with (
            tc.tile_pool(name="const", bufs=1) as constp,
            tc.tile_pool(name="xtp", bufs=3) as xtp,
            tc.tile_pool(name="qkv", bufs=1) as qkvp,
            tc.tile_pool(name="vts", bufs=2) as vtsp,
            tc.tile_pool(name="esc", bufs=4) as escp,
            tc.tile_pool(name="ctx", bufs=2) as ctxp,
            tc.tile_pool(name="small", bufs=2) as smallp,
            tc.tile_pool(name="bsb", bufs=2) as bsbp,
            tc.tile_pool(name="psS", bufs=3, space="PSUM") as psS,
            tc.tile_pool(name="psC", bufs=2, space="PSUM") as psC,
        ):
            # ---- constants ----
            wq_sb = constp.tile([P, DC, P], f32r, tag="wq")
            wk_sb = constp.tile([P, DC, P], f32r, tag="wk")
            wv_sb = constp.tile([P, DC, P], f32r, tag="wv")
            bq_sb = constp.tile([P, 1], f32, tag="bq")
            nc.sync.dma_start(bq_sb, bqd)
            bk_sb = constp.tile([P, 1], f32, tag="bk")
            nc.sync.dma_start(bk_sb, bkd)
            bv_sb = constp.tile([P, 1], f32, tag="bv")
            nc.sync.dma_start(bv_sb, bvd)
            ident_f = constp.tile([P, P], f32, tag="identf")
            make_identity(nc, ident_f)
            ident = constp.tile([P, P], f32r, tag="ident")
            nc.vector.tensor_copy(ident, ident_f)
            ones_f32 = constp.tile([P, 512], f32, tag="ones_f32")
            nc.vector.memset(ones_f32, 1.0)
            ones_t = constp.tile([P, 512], f32r, tag="ones")
            nc.vector.tensor_copy(ones_t, ones_f32)
            wo_sb = constp.tile([P, D], f32r, tag="wo")

            # ---- per-batch persistent tiles ----
            qt_sb = [
                qkvp.tile([P, T], f16, tag=f"qt{b}", name=f"qt{b}")
                for b in range(NB)
            ]
            kt_sb = [
                qkvp.tile([P, T], f16, tag=f"kt{b}", name=f"kt{b}")
                for b in range(NB)
            ]
            # V natural per batch per head, 65-wide blocks: [V(64) | ones]
            va_sb = [
                qkvp.tile([P, KT * 65], f16, tag=f"va{b}", name=f"va{b}")
                for b in range(NB)
            ]
            vb_sb = [
                qkvp.tile([P, KT * 65], f16, tag=f"vb{b}", name=f"vb{b}")
                for b in range(NB)
            ]
            ones_col = ones_f32[:, 0:KT].rearrange("p (k one) -> p k one", one=1)
            for b in range(NB):
                nc.vector.tensor_copy(
                    va_sb[b].rearrange("p (k c) -> p k c", c=65)[:, :, 64:65],
                    ones_col,
                )
                nc.vector.tensor_copy(
                    vb_sb[b].rearrange("p (k c) -> p k c", c=65)[:, :, 64:65],
                    ones_col,
                )

            xt_r = xt.rearrange("(c p) t -> p c t", p=P)

            for _rep in range(reps):

                def load_xtile(tch):
                    xtile = xtp.tile([P, DC, 512], f32r, tag="xt", name="xtile")
                    nc.sync.dma_start(xtile, xt_r[:, :, ds(tch * 512, 512)])
                    return xtile

                def proj_w(tch, xtile, w_sb, b_sb, dst, half):
                    # half 0/1: 4 contraction chunks each; half 1 closes the
                    # accumulation group and writes dst
                    ps_name = f"ps{tch}_{id(w_sb) % 97}"
                    if half == 0:
                        ps = psS.tile([P, 512], f32, tag="sc", name="psw")
                        _proj_ps[(tch, id(w_sb))] = ps
                        for c in range(4):
                            nc.tensor.matmul(
                                ps, w_sb[:, c], xtile[:, c],
                                start=(c == 0), stop=False,
                            )
                    else:
                        ps = _proj_ps.pop((tch, id(w_sb)))
                        for c in range(4, DC):
                            nc.tensor.matmul(
                                ps, w_sb[:, c], xtile[:, c],
                                start=False, stop=(c == DC - 1),
                            )
                        nc.vector.tensor_scalar_add(dst, ps, b_sb)

                def proj_v_tail(tch, vts, half):
                    # transpose VT -> V natural; 2 t-tiles per half
                    b = tch // 4
                    for tt in (0, 1) if half == 0 else (2, 3):
                        ktile = (tch % 4) * 4 + tt
                        pvt = psS.tile([P, P], f32r, tag="sc", name="pvt")
                        nc.tensor.transpose(pvt, vts[:, ds(tt * P, P)], ident)
                        nc.vector.tensor_copy(
                            va_sb[b][:, ds(ktile * 65, DK)], pvt[:, 0:DK]
                        )
                        nc.vector.tensor_copy(
                            vb_sb[b][:, ds(ktile * 65, DK)], pvt[:, DK:P]
                        )

                _proj_ps = {}

                def proj_k(tch, xtile):
                    b = tch // 4
                    tloc = (tch % 4) * 512
                    dst = kt_sb[b][:, ds(tloc, 512)]
                    proj_w(tch, xtile, wk_sb, bk_sb, dst, 0)
                    proj_w(tch, xtile, wk_sb, bk_sb, dst, 1)

                def proj_v(tch, xtile):
                    vts = vtsp.tile([P, 512], f32r, tag="vts", name="vts")
                    proj_w(tch, xtile, wv_sb, bv_sb, vts, 0)
                    proj_w(tch, xtile, wv_sb, bv_sb, vts, 1)
                    proj_v_tail(tch, vts, 0)
                    proj_v_tail(tch, vts, 1)

                def proj_q_thunks(tch, xtile):
                    b = tch // 4
                    tloc = (tch % 4) * 512
                    dst = qt_sb[b][:, ds(tloc, 512)]
                    return [
                        lambda: proj_w(tch, xtile, wq_sb, bq_sb, dst, 0),
                        lambda: proj_w(tch, xtile, wq_sb, bq_sb, dst, 1),
                    ]

                def proj_kv_thunks(tch, xtile):
                    b = tch // 4
                    tloc = (tch % 4) * 512
                    kdst = kt_sb[b][:, ds(tloc, 512)]
                    vts = vtsp.tile([P, 512], f32r, tag="vts", name="vts")
                    return [
                        lambda: proj_w(tch, xtile, wk_sb, bk_sb, kdst, 0),
                        lambda: proj_w(tch, xtile, wk_sb, bk_sb, kdst, 1),
                        lambda: proj_w(tch, xtile, wv_sb, bv_sb, vts, 0),
                        lambda: proj_w(tch, xtile, wv_sb, bv_sb, vts, 1),
                        lambda: proj_v_tail(tch, vts, 0),
                        lambda: proj_v_tail(tch, vts, 1),
                    ]

                # pipelined finalize: stage1 (bcast+normalize) and stage2
                # (out-projection) of the previous chunk are emitted inside
                # the current chunk's kt loop to keep the PE stream dense.
                def fin_stage1(st):
                    b, qch, ua, ub = st
                    rf = smallp.tile([P, 1024], f32, tag="recipf", name="rf")
                    nc.vector.reciprocal(rf[64:65, 0:512], ua[64:65, :])
                    nc.vector.reciprocal(rf[64:65, 512:1024], ub[64:65, :])
                    rr = smallp.tile([P, 1024], f32r, tag="recip", name="rr")
                    nc.vector.tensor_copy(rr[64:65, :], rf[64:65, :])
                    bc = psS.tile([P, 1024], f32, tag="sc", name="bc")
                    nc.tensor.matmul(
                        bc[0:DK, 0:512], ones_t[64:65, 0:DK], rr[64:65, 0:512],
                        start=True, stop=True, tile_position=(64, 0),
                    )
                    nc.tensor.matmul(
                        bc[0:DK, 512:1024], ones_t[64:65, 0:DK],
                        rr[64:65, 512:1024],
                        start=True, stop=True, tile_position=(64, 0),
                    )
                    bc_sb = bsbp.tile([DK, 1024], f32, tag="bcs", name="bc_sb")
                    nc.vector.tensor_copy(bc_sb, bc[0:DK, :])
                    ctq = ctxp.tile([P, 512], f32r, tag="ctq", name="ctq")
                    nc.vector.tensor_mul(
                        ctq[0:DK, :], ua[0:DK, :], bc_sb[:, 0:512]
                    )
                    tmpb = bsbp.tile([DK, 512], f32r, tag="tmpb", name="tmpb")
                    nc.vector.tensor_mul(tmpb, ub[0:DK, :], bc_sb[:, 512:1024])
                    nc.sync.dma_start(ctq[DK:P, :], tmpb)
                    return ctq

                def fin_stage2(st, ctq, tts):
                    b, qch, ua, ub = st
                    q0 = qch * 512
                    for tt in tts:
                        po = psS.tile([P, 1024], f32, tag="sc", name="po")
                        nc.tensor.matmul(
                            po[:, 0:512],
                            ctq[:, ds(tt * P, P)], wo_sb[:, 0:512],
                            start=True, stop=True,
                        )
                        nc.tensor.matmul(
                            po[:, 512:1024],
                            ctq[:, ds(tt * P, P)], wo_sb[:, 512:1024],
                            start=True, stop=True,
                        )
                        po_sb = escp.tile([P, 1024], f32, tag="posb", name="po_sb", bufs=3)
                        nc.vector.tensor_copy(po_sb, po)
                        r0 = b * T + q0 + tt * P
                        nc.sync.dma_start(out[r0 : r0 + P, :], po_sb)

                pending = {"st": None, "ctq": None}

                work_q = []

                def drain_pending(kt):
                    # interleave previous chunk's finalize into this kt loop;
                    # stage1 late enough that the DVE reciprocal chain has
                    # completed (it starts right after the previous chunk)
                    if pending["st"] is not None:
                        if kt == 8:
                            pending["ctq"] = fin_stage1(pending["st"])
                            return
                        elif kt == 12:
                            fin_stage2(pending["st"], pending["ctq"], (0, 1))
                            return
                        elif kt == 15:
                            fin_stage2(pending["st"], pending["ctq"], (2, 3))
                            pending["st"] = None
                            return
                    if kt in (1, 2, 3, 4, 5, 6, 7, 9, 10, 11, 13, 14) \
                            and work_q:
                        work_q.pop(0)()

                def attn_chunk(b, qch):
                    q0 = qch * 512
                    cxa = psC.tile([65, 512], f32, tag="cx", name="cxa")
                    cxb = psC.tile([65, 512], f32, tag="cx", name="cxb")
                    for kt in range(KT):
                        sc = psS.tile([P, 1024], f32, tag="sc", name="sc")
                        nc.tensor.matmul(
                            sc[:, 0:512],
                            kt_sb[b][0:DK, ds(kt * P, P)],
                            qt_sb[b][0:DK, ds(q0, 512)],
                            start=True, stop=True,
                        )
                        nc.tensor.matmul(
                            sc[:, 512:1024],
                            kt_sb[b][DK:P, ds(kt * P, P)],
                            qt_sb[b][DK:P, ds(q0, 512)],
                            start=True, stop=True,
                            tile_position=(64, 0),
                        )
                        esc = escp.tile([P, 1024], f16, tag="esc", name="esc")
                        nc.scalar.activation(esc, sc, Exp, scale=0.125)
                        nc.tensor.matmul(
                            cxa,
                            va_sb[b][:, ds(kt * 65, 65)],
                            esc[:, 0:512],
                            start=(kt == 0), stop=(kt == KT - 1),
                        )
                        nc.tensor.matmul(
                            cxb,
                            vb_sb[b][:, ds(kt * 65, 65)],
                            esc[:, 512:1024],
                            start=(kt == 0), stop=(kt == KT - 1),
                        )
                        drain_pending(kt)
                    ua = bsbp.tile([65, 512], f32, tag="ua", name="ua")
                    nc.vector.tensor_copy(ua, cxa)
                    ub = bsbp.tile([65, 512], f32, tag="ub", name="ub")
                    nc.vector.tensor_copy(ub, cxb)
                    return (b, qch, ua, ub)

                if _rep == 0:
                    nc.sync.dma_start(
                        wk_sb, wk.rearrange("(c p) e -> p c e", p=P)
                    )
                    nc.sync.dma_start(
                        wv_sb, wv.rearrange("(c p) e -> p c e", p=P)
                    )
                    nc.sync.dma_start(
                        wq_sb, wq.rearrange("(c p) e -> p c e", p=P)
                    )
                # b0 fully upfront
                for tch in range(4):
                    xti = load_xtile(tch)
                    proj_k(tch, xti)
                    proj_v(tch, xti)
                    for th in proj_q_thunks(tch, xti):
                        th()
                    if tch == 0 and _rep == 0:
                        nc.sync.dma_start(wo_sb, wo)
                # defer all of b1's projections into b0's attention
                xt_b1 = {}

                def mk_load(tch):
                    def th():
                        xt_b1[tch] = load_xtile(tch)
                    return th

                def mk_body(tch):
                    out = []
                    for i in range(6):
                        def th(tch=tch, i=i):
                            xti = xt_b1[tch]
                            if "thunks" not in xt_b1.setdefault(
                                f"t{tch}", {}
                            ):
                                xt_b1[f"t{tch}"]["thunks"] = (
                                    proj_kv_thunks(tch, xti)
                                    + proj_q_thunks(tch, xti)
                                )
                            xt_b1[f"t{tch}"]["thunks"][i]()
                        out.append(th)
                    for i in range(2):
                        def th2(tch=tch, i=i):
                            xt_b1[f"t{tch}"]["thunks"][6 + i]()
                        out.append(th2)
                    return out

                work_q.append(mk_load(4))
                work_q.append(mk_load(5))
                work_q.extend(mk_body(4))
                work_q.append(mk_load(6))
                work_q.extend(mk_body(5))
                work_q.append(mk_load(7))
                work_q.extend(mk_body(6))
                work_q.extend(mk_body(7))
                for qch in range(4):
                    pending["st"] = attn_chunk(0, qch)
                while work_q:
                    work_q.pop(0)()
                for qch in range(4):
                    pending["st"] = attn_chunk(1, qch)
                ctq = fin_stage1(pending["st"])
                fin_stage2(pending["st"], ctq, (0, 1, 2, 3))

    nc.compile()
    return nc


def _get_nc(reps=1):
    key = f"nc{reps}"
    if key not in _CACHE:
        _CACHE[key] = _build(reps)
    return _CACHE[key]


def kernel(x, Wq, bq, Wk, bk, Wv, bv, Wo, bo):
    from concourse.bass_utils import run_bass_kernel_spmd

    x = np.asarray(x, dtype=np.float32)
    Wq = np.asarray(Wq, dtype=np.float32)
    Wk = np.asarray(Wk, dtype=np.float32)
    Wv = np.asarray(Wv, dtype=np.float32)
    Wo = np.asarray(Wo, dtype=np.float32)
    bq = np.asarray(bq, dtype=np.float32)
    bk = np.asarray(bk, dtype=np.float32)
    bv = np.asarray(bv, dtype=np.float32)
    bo = np.asarray(bo, dtype=np.float32)

    B, Tl, Dl = x.shape
    xt = np.ascontiguousarray(x.reshape(B * Tl, Dl).T)

    in_maps = []
    for c in range(NCORES):
        sl = slice(c * P, (c + 1) * P)
        in_maps.append(
            {
                "xt": xt,
                "wq": np.ascontiguousarray(Wq[sl, :].T),
                "wk": np.ascontiguousarray(Wk[sl, :].T),
                "wv": np.ascontiguousarray(Wv[sl, :].T),
                "wo": np.ascontiguousarray(Wo[:, sl].T),
                "bq": np.ascontiguousarray(bq[sl].reshape(P, 1)),
                "bk": np.ascontiguousarray(bk[sl].reshape(P, 1)),
                "bv": np.ascontiguousarray(bv[sl].reshape(P, 1)),
            }
        )

    nc = _get_nc()
    _CACHE["in_maps"] = in_maps
    res = run_bass_kernel_spmd(nc, in_maps, core_ids=list(range(NCORES)))
    acc = res.results[0]["out"].astype(np.float32)
    for c in range(1, NCORES):
        acc = acc + res.results[c]["out"]
    acc = acc + bo[None, :]
    return acc.reshape(B, Tl, Dl).astype(np.float32)



# revision 12
# speedup vs baseline: 1.0610x; 1.0610x over previous
"""Trainium2 Bass kernel for nn_MultiHeadAttention (B=2, T=2048, D=1024, H=16).

Sharding: 8 cores; core c owns head pair (2c, 2c+1) = output-channel slice
[c*128, (c+1)*128) of Wq/Wk/Wv columns and Wo rows (tensor parallel), both
batches. Host pre-transposes x and weight slices (cast to f16); each core
computes a partial output projection over its 128 ctx channels in f16; host
sums the 8 partials in f32 (replaces the all-reduce) and adds bo.

v2: the kernel is organized as one dense exp stream on the scalar engine
(the binding engine at ~1.0us per [128,1024] tile, 128 tiles) with every
other piece of work (projections, V transposes, out-projection, softmax
normalization) scheduled into per-ktile drain slots under it. Attention
starts as soon as K(b0,tch0)+Q(tch0)+V(tch0) are projected (~10us); all
remaining projections fill PE slack inside the attention phase.
"""

import numpy as np

P = 128
D = 1024
BT = 4096
T = 2048
NB = 2
DC = 8    # D chunks of 128
KT = 16   # 128-wide k-tiles per batch
NCORES = 8
DK = 64

_CACHE = {}


def _build(reps=1, debug=False):
    import concourse.bass as bass
    import concourse.tile as tile
    from concourse import bacc, mybir
    from concourse.masks import make_identity

    f32 = mybir.dt.float32
    f16 = mybir.dt.float16
    Exp = mybir.ActivationFunctionType.Exp
    ds = bass.ds

    nc = bacc.Bacc("TRN2", target_bir_lowering=False, debug=False)

    xt = nc.dram_tensor("xt", [D, BT], f16, kind="ExternalInput").ap()
    wq = nc.dram_tensor("wq", [D, P], f16, kind="ExternalInput").ap()
    wk = nc.dram_tensor("wk", [D, P], f16, kind="ExternalInput").ap()
    wv = nc.dram_tensor("wv", [D, P], f16, kind="ExternalInput").ap()
    wo = nc.dram_tensor("wo", [P, D], f16, kind="ExternalInput").ap()
    bqd = nc.dram_tensor("bq", [P, 1], f32, kind="ExternalInput").ap()
    bkd = nc.dram_tensor("bk", [P, 1], f32, kind="ExternalInput").ap()
    bvd = nc.dram_tensor("bv", [P, 1], f32, kind="ExternalInput").ap()
    out = nc.dram_tensor("out", [BT, D], f16, kind="ExternalOutput").ap()
    dbg = {}
    if debug:
        dbg["qt"] = nc.dram_tensor("dbg_qt", [P, T], f16, kind="ExternalOutput").ap()
        dbg["kt"] = nc.dram_tensor("dbg_kt", [P, T], f16, kind="ExternalOutput").ap()
        dbg["va"] = nc.dram_tensor("dbg_va", [P, KT * 65], f16, kind="ExternalOutput").ap()
        dbg["vb"] = nc.dram_tensor("dbg_vb", [P, KT * 65], f16, kind="ExternalOutput").ap()
        dbg["esc"] = nc.dram_tensor("dbg_esc", [P, 1024], f16, kind="ExternalOutput").ap()
        dbg["ua"] = nc.dram_tensor("dbg_ua", [65, 512], f32, kind="ExternalOutput").ap()
        dbg["ub"] = nc.dram_tensor("dbg_ub", [65, 512], f32, kind="ExternalOutput").ap()
        dbg["rr"] = nc.dram_tensor("dbg_rr", [1, 1024], f16, kind="ExternalOutput").ap()
        dbg["ctq"] = nc.dram_tensor("dbg_ctq", [P, 512], f16, kind="ExternalOutput").ap()

    with tile.TileContext(nc) as tc:
        with (
            tc.tile_pool(name="const", bufs=1) as constp,
            tc.tile_pool(name="xtp", bufs=4) as xtp,
            tc.tile_pool(name="qkv", bufs=1) as qkvp,
            tc.tile_pool(name="vts", bufs=2) as vtsp,
            tc.tile_pool(name="esc", bufs=3) as escp,
            tc.tile_pool(name="ctq", bufs=2) as ctqp,
            tc.tile_pool(name="small", bufs=2) as smallp,
            tc.tile_pool(name="bsb", bufs=2) as bsbp,
            tc.tile_pool(name="posb", bufs=3) as posbp,
            # PSUM: sc 2x2 banks + cx 2x1 banks + flex 2x1 banks = 8
            tc.tile_pool(name="psS", bufs=2, space="PSUM") as psS,
            tc.tile_pool(name="psC", bufs=2, space="PSUM") as psC,
            tc.tile_pool(name="psF", bufs=2, space="PSUM") as psF,
        ):
            # ---- constants ----
            wq_sb = constp.tile([P, DC, P], f16, tag="wq")
            wk_sb = constp.tile([P, DC, P], f16, tag="wk")
            wv_sb = constp.tile([P, DC, P], f16, tag="wv")
            bq_sb = constp.tile([P, 1], f32, tag="bq")
            nc.sync.dma_start(bq_sb, bqd)
            bk_sb = constp.tile([P, 1], f32, tag="bk")
            nc.sync.dma_start(bk_sb, bkd)
            bv_sb = constp.tile([P, 1], f32, tag="bv")
            nc.sync.dma_start(bv_sb, bvd)
            ident_f = constp.tile([P, P], f32, tag="identf")
            make_identity(nc, ident_f)
            ident = constp.tile([P, P], f16, tag="ident")
            nc.vector.tensor_copy(ident, ident_f)
            ones16 = constp.tile([P, 512], f16, tag="ones16")
            nc.vector.memset(ones16, 1.0)
            wo_sb = constp.tile([P, D], f16, tag="wo")

            nc.sync.dma_start(wk_sb, wk.rearrange("(c p) e -> p c e", p=P))
            nc.sync.dma_start(wq_sb, wq.rearrange("(c p) e -> p c e", p=P))
            nc.sync.dma_start(wv_sb, wv.rearrange("(c p) e -> p c e", p=P))
            nc.sync.dma_start(wo_sb, wo)

            # ---- per-batch persistent tiles ----
            qt_sb = [
                qkvp.tile([P, T], f16, tag=f"qt{b}", name=f"qt{b}")
                for b in range(NB)
            ]
            kt_sb = [
                qkvp.tile([P, T], f16, tag=f"kt{b}", name=f"kt{b}")
                for b in range(NB)
            ]
            # V natural per batch, 65-wide blocks per ktile: [V_a(64)|1] and
            # [V_b(64)|1]; the ones column accumulates the softmax denominator
            va_sb = [
                qkvp.tile([P, KT * 65], f16, tag=f"va{b}", name=f"va{b}")
                for b in range(NB)
            ]
            vb_sb = [
                qkvp.tile([P, KT * 65], f16, tag=f"vb{b}", name=f"vb{b}")
                for b in range(NB)
            ]
            ones_col = ones16[:, 0:KT].rearrange("p (k one) -> p k one", one=1)
            for b in range(NB):
                nc.vector.tensor_copy(
                    va_sb[b].rearrange("p (k c) -> p k c", c=65)[:, :, 64:65],
                    ones_col,
                )
                nc.vector.tensor_copy(
                    vb_sb[b].rearrange("p (k c) -> p k c", c=65)[:, :, 64:65],
                    ones_col,
                )

            xt_r = xt.rearrange("(c p) t -> p c t", p=P)

            xtiles = {}

            def load_x(tch):
                def th():
                    t0 = xtp.tile([P, DC, 512], f16, tag="xt", name=f"x{tch}")
                    nc.sync.dma_start(t0, xt_r[:, :, ds(tch * 512, 512)])
                    xtiles[tch] = t0
                return th

            _proj_ps = {}

            def proj_half(tch, w_sb, b_sb, dst, half):
                # half 0: open PSUM accumulation, 4 contraction chunks;
                # half 1: 4 more chunks, close group, evict (bias+cast f16)
                def th():
                    if half == 0:
                        ps = psF.tile([P, 512], f32, tag="fx", name="pj")
                        _proj_ps[(tch, id(w_sb))] = ps
                        for c in range(4):
                            nc.tensor.matmul(
                                ps, w_sb[:, c], xtiles[tch][:, c],
                                start=(c == 0), stop=False,
                            )
                    else:
                        ps = _proj_ps.pop((tch, id(w_sb)))
                        for c in range(4, DC):
                            nc.tensor.matmul(
                                ps, w_sb[:, c], xtiles[tch][:, c],
                                start=False, stop=(c == DC - 1),
                            )
                        nc.vector.tensor_scalar_add(dst, ps, b_sb)
                return th

            def K(tch, half):
                b = tch // 4
                dst = kt_sb[b][:, ds((tch % 4) * 512, 512)]
                return proj_half(tch, wk_sb, bk_sb, dst, half)

            def Q(tch, half):
                b = tch // 4
                dst = qt_sb[b][:, ds((tch % 4) * 512, 512)]
                return proj_half(tch, wq_sb, bq_sb, dst, half)

            _vts = {}

            def V(tch, half):
                def th():
                    if half == 0:
                        _vts[tch] = vtsp.tile([P, 512], f32, tag="vts",
                                              name=f"v{tch}")
                    proj_half(tch, wv_sb, bv_sb, _vts[tch], half)()
                return th

            def Vt(tch, half):
                # transpose VT -> V natural; 2 t-tiles of 128 per half.
                # Both transposes go into one PSUM tile back-to-back, then
                # the DVE copies drain it (keeps the PE stream dense).
                def th():
                    b = tch // 4
                    vts = _vts[tch]
                    tts = (0, 1) if half == 0 else (2, 3)
                    pvt = psF.tile([P, 256], f32, tag="fx", name="pvt")
                    for j, tt in enumerate(tts):
                        nc.tensor.transpose(
                            pvt[:, ds(j * P, P)], vts[:, ds(tt * P, P)], ident_f
                        )
                    for j, tt in enumerate(tts):
                        ktile = (tch % 4) * 4 + tt
                        nc.vector.tensor_copy(
                            va_sb[b][:, ds(ktile * 65, DK)],
                            pvt[:, ds(j * P, DK)],
                        )
                        nc.vector.tensor_copy(
                            vb_sb[b][:, ds(ktile * 65, DK)],
                            pvt[:, ds(j * P + DK, DK)],
                        )
                return th

            # ---- finalize thunks for a finished chunk ----
            def fin_make(st):
                b, qch, ua, ub = st
                box = {}

                def R(which):
                    def th():
                        if which == 0:
                            box["rr"] = smallp.tile(
                                [P, 1024], f16, tag="rr", name="rr")
                        src = ua if which == 0 else ub
                        with nc.allow_low_precision("softmax denom recip"):
                            nc.vector.reciprocal(
                                box["rr"][64:65, ds(which * 512, 512)],
                                src[64:65, :],
                            )
                        if debug and b == 0 and qch == 0 and which == 1:
                            nc.sync.dma_start(dbg["rr"], box["rr"][64:65, :])
                    return th

                def bc(which):
                    def th():
                        rr = box["rr"]
                        ps = psF.tile([P, 512], f32, tag="fx", name="bc")
                        nc.tensor.matmul(
                            ps[0:DK, :], ones16[64:65, 0:DK],
                            rr[64:65, ds(which * 512, 512)],
                            start=True, stop=True, tile_position=(64, 0),
                        )
                        box[f"bc{which}"] = ps
                    return th

                def mul_a():
                    ctq = ctqp.tile([P, 512], f16, tag="ctq", name="ctq")
                    nc.vector.tensor_mul(
                        ctq[0:DK, :], ua[0:DK, :], box["bc0"][0:DK, :])
                    box["ctq"] = ctq

                def mul_b():
                    tmpb = bsbp.tile([DK, 512], f16, tag="tmpb", name="tmpb")
                    nc.vector.tensor_mul(tmpb, ub[0:DK, :], box["bc1"][0:DK, :])
                    nc.sync.dma_start(box["ctq"][DK:P, :], tmpb)

                def op(tt):
                    def th():
                        ctq = box["ctq"]
                        if debug and b == 0 and qch == 0 and tt == 0:
                            nc.sync.dma_start(dbg["ctq"], ctq)
                        po_sb = posbp.tile([P, 1024], f16, tag="po", name="po_sb")
                        pos = []
                        for half in range(2):
                            po = psF.tile([P, 512], f32, tag="fx", name="po")
                            nc.tensor.matmul(
                                po, ctq[:, ds(tt * P, P)],
                                wo_sb[:, ds(half * 512, 512)],
                                start=True, stop=True,
                            )
                            pos.append(po)
                        for half in range(2):
                            nc.vector.tensor_copy(
                                po_sb[:, ds(half * 512, 512)], pos[half])
                        r0 = b * T + qch * 512 + tt * P
                        nc.sync.dma_start(out[r0: r0 + P, :], po_sb)
                    return th

                return [R(0), R(1), bc(0), bc(1), mul_a, mul_b,
                        op(0), op(1), op(2), op(3)]

            # ---- attention chunk: 16 kt slots, scores one ahead of ctx ----
            def attn_chunk(b, qch, drains):
                q0 = qch * 512
                cxa = psC.tile([65, 512], f32, tag="cx", name="cxa")
                cxb = psC.tile([65, 512], f32, tag="cx", name="cxb")
                escs = {}

                def ctx(kt):
                    e = escs.pop(kt)
                    nc.tensor.matmul(
                        cxa, va_sb[b][:, ds(kt * 65, 65)], e[:, 0:512],
                        start=(kt == 0), stop=(kt == KT - 1),
                    )
                    nc.tensor.matmul(
                        cxb, vb_sb[b][:, ds(kt * 65, 65)], e[:, 512:1024],
                        start=(kt == 0), stop=(kt == KT - 1),
                    )

                for kt in range(KT):
                    sc = psS.tile([P, 1024], f32, tag="sc", name="sc")
                    nc.tensor.matmul(
                        sc[:, 0:512],
                        kt_sb[b][0:DK, ds(kt * P, P)],
                        qt_sb[b][0:DK, ds(q0, 512)],
                        start=True, stop=True,
                    )
                    nc.tensor.matmul(
                        sc[:, 512:1024],
                        kt_sb[b][DK:P, ds(kt * P, P)],
                        qt_sb[b][DK:P, ds(q0, 512)],
                        start=True, stop=True,
                        tile_position=(64, 0),
                    )
                    esc = escp.tile([P, 1024], f16, tag="esc", name="esc")
                    nc.scalar.activation(esc, sc, Exp, scale=0.125)
                    escs[kt] = esc
                    if debug and b == 0 and qch == 0 and kt == 0:
                        nc.sync.dma_start(dbg["esc"], esc)
                    if kt > 0:
                        ctx(kt - 1)
                    for th in drains[kt]:
                        th()
                ctx(KT - 1)
                ua = bsbp.tile([65, 512], f32, tag="ua", name="ua")
                nc.vector.tensor_copy(ua, cxa)
                ub = bsbp.tile([65, 512], f32, tag="ub", name="ub")
                nc.vector.tensor_copy(ub, cxb)
                if debug and b == 0 and qch == 0:
                    nc.sync.dma_start(dbg["ua"], ua)
                    nc.sync.dma_start(dbg["ub"], ub)
                return (b, qch, ua, ub)

            # ---- schedule ----
            # head: load b0 x tiles, project K0/Q0/V0, transpose V0
            for tch in range(4):
                load_x(tch)()
            K(0, 0)(); K(0, 1)()
            Q(0, 0)(); Q(0, 1)()
            V(0, 0)(); V(0, 1)()
            Vt(0, 0)(); Vt(0, 1)()

            def sched(*slots):
                d = [[] for _ in range(KT)]
                for i, s in enumerate(slots):
                    if s:
                        d[i] = list(s) if isinstance(s, (list, tuple)) else [s]
                return d

            fin = {}

            # chunk (0,0): remaining b0 projections
            fin[(0, 0)] = attn_chunk(0, 0, sched(
                [K(1, 0), K(1, 1)], [V(1, 0), V(1, 1)], [Vt(1, 0), Vt(1, 1)],
                [K(2, 0), K(2, 1)], [V(2, 0), V(2, 1)], [Vt(2, 0), Vt(2, 1)],
                K(3, 0), K(3, 1), V(3, 0), V(3, 1), Vt(3, 0), Vt(3, 1),
                Q(1, 0), Q(1, 1), None, None,
            ))
            f = fin_make(fin[(0, 0)])
            fin[(0, 1)] = attn_chunk(0, 1, sched(
                [load_x(4), f[0]], [f[1], Q(2, 0)], Q(2, 1), f[2], f[3], f[4],
                f[5], f[6], f[7], f[8], f[9], K(4, 0), K(4, 1),
                [load_x(5), V(4, 0)], V(4, 1), [Vt(4, 0), Vt(4, 1)],
            ))
            f = fin_make(fin[(0, 1)])
            fin[(0, 2)] = attn_chunk(0, 2, sched(
                [Q(3, 0), f[0]], [Q(3, 1), f[1]], Q(4, 0), Q(4, 1), f[2], f[3],
                f[4], f[5], f[6], f[7], f[8], f[9], [load_x(6), K(5, 0)],
                K(5, 1), [V(5, 0), V(5, 1)], [Vt(5, 0), Vt(5, 1)],
            ))
            f = fin_make(fin[(0, 2)])
            fin[(0, 3)] = attn_chunk(0, 3, sched(
                f[0], f[1], f[2], f[3], f[4], f[5], f[6], f[7], f[8], f[9],
                [load_x(7), K(6, 0)], K(6, 1), V(6, 0), V(6, 1),
                [Vt(6, 0), Vt(6, 1)], None,
            ))
            f = fin_make(fin[(0, 3)])
            fin[(1, 0)] = attn_chunk(1, 0, sched(
                [K(7, 0), f[0]], [K(7, 1), f[1]], V(7, 0), V(7, 1),
                Vt(7, 0), Vt(7, 1), f[2], f[3], f[4], f[5], Q(5, 0), Q(5, 1),
                f[6], f[7], f[8], f[9],
            ))
            f = fin_make(fin[(1, 0)])
            fin[(1, 1)] = attn_chunk(1, 1, sched(
                f[0], f[1], f[2], f[3], f[4], f[5], Q(6, 0), Q(6, 1),
                f[6], f[7], f[8], f[9], None, None, None, None,
            ))
            f = fin_make(fin[(1, 1)])
            fin[(1, 2)] = attn_chunk(1, 2, sched(
                f[0], f[1], f[2], f[3], f[4], f[5], Q(7, 0), Q(7, 1),
                f[6], f[7], f[8], f[9], None, None, None, None,
            ))
            f = fin_make(fin[(1, 2)])
            fin[(1, 3)] = attn_chunk(1, 3, sched(
                f[0], f[1], f[2], f[3], f[4], f[5], f[6], f[7], f[8], f[9],
                None, None, None, None, None, None,
            ))
            # tail: finalize the last chunk
            for th in fin_make(fin[(1, 3)]):
                th()
            if debug:
                nc.sync.dma_start(dbg["qt"], qt_sb[0])
                nc.sync.dma_start(dbg["kt"], kt_sb[0])
                nc.sync.dma_start(dbg["va"], va_sb[0])
                nc.sync.dma_start(dbg["vb"], vb_sb[0])

    nc.compile()
    return nc


def _get_nc(reps=1, debug=False):
    key = f"nc{reps}_{debug}"
    if key not in _CACHE:
        _CACHE[key] = _build(reps, debug=debug)
    return _CACHE[key]


def kernel(x, Wq, bq, Wk, bk, Wv, bv, Wo, bo):
    from concourse.bass_utils import run_bass_kernel_spmd

    x = np.asarray(x, dtype=np.float32)
    Wq = np.asarray(Wq, dtype=np.float32)
    Wk = np.asarray(Wk, dtype=np.float32)
    Wv = np.asarray(Wv, dtype=np.float32)
    Wo = np.asarray(Wo, dtype=np.float32)
    bq = np.asarray(bq, dtype=np.float32)
    bk = np.asarray(bk, dtype=np.float32)
    bv = np.asarray(bv, dtype=np.float32)
    bo = np.asarray(bo, dtype=np.float32)

    B, Tl, Dl = x.shape
    xt = np.ascontiguousarray(x.reshape(B * Tl, Dl).T.astype(np.float16))

    in_maps = []
    for c in range(NCORES):
        sl = slice(c * P, (c + 1) * P)
        in_maps.append(
            {
                "xt": xt,
                "wq": np.ascontiguousarray(Wq[sl, :].T.astype(np.float16)),
                "wk": np.ascontiguousarray(Wk[sl, :].T.astype(np.float16)),
                "wv": np.ascontiguousarray(Wv[sl, :].T.astype(np.float16)),
                "wo": np.ascontiguousarray(Wo[:, sl].T.astype(np.float16)),
                "bq": np.ascontiguousarray(bq[sl].reshape(P, 1)),
                "bk": np.ascontiguousarray(bk[sl].reshape(P, 1)),
                "bv": np.ascontiguousarray(bv[sl].reshape(P, 1)),
            }
        )

    nc = _get_nc()
    _CACHE["in_maps"] = in_maps
    res = run_bass_kernel_spmd(nc, in_maps, core_ids=list(range(NCORES)))
    acc = res.results[0]["out"].astype(np.float32)
    for c in range(1, NCORES):
        acc = acc + res.results[c]["out"].astype(np.float32)
    acc = acc + bo[None, :]
    return acc.reshape(B, Tl, Dl).astype(np.float32)


# revision 13
# speedup vs baseline: 1.2548x; 1.1827x over previous
"""Trainium2 Bass kernel for nn_MultiHeadAttention (B=2, T=2048, D=1024, H=16).

Sharding: 8 cores; core c owns head pair (2c, 2c+1) = output-channel slice
[c*128, (c+1)*128) of Wq/Wk/Wv columns and Wo rows (tensor parallel), both
batches. Host pre-transposes x and weight slices (cast to f16); each core
computes a partial output projection over its 128 ctx channels in f16; host
sums the 8 partials in f32 (replaces the all-reduce) and adds bo.

The kernel is one dense exp stream on the scalar engine (the binding engine
at ~1.0us per [128,1024] score tile, 128 tiles) with all other work
(projections, V transposes, out-projection, softmax normalization) scheduled
into per-ktile drain slots under it. The 8 attention chunks form a single
flat 128-slot pipeline: scores(kt) -> exp(kt) -> ctx(kt) with scores one
slot ahead of ctx, continuing seamlessly across chunk boundaries; the
previous chunk's finalize (reciprocal of the softmax denominators staged to
partition 0 via a small DMA, PE broadcast, normalize, out-projection) is
drained through the following chunk's slots.
"""

import numpy as np

P = 128
D = 1024
BT = 4096
T = 2048
NB = 2
DC = 8    # D chunks of 128
KT = 16   # 128-wide k-tiles per batch
NCORES = 8
DK = 64

_CACHE = {}


def _build(reps=1, debug=False):
    import concourse.bass as bass
    import concourse.tile as tile
    from concourse import bacc, mybir
    from concourse.masks import make_identity

    f32 = mybir.dt.float32
    f16 = mybir.dt.float16
    Exp = mybir.ActivationFunctionType.Exp
    ds = bass.ds

    nc = bacc.Bacc("TRN2", target_bir_lowering=False, debug=False)

    xt = nc.dram_tensor("xt", [D, BT], f16, kind="ExternalInput").ap()
    wq = nc.dram_tensor("wq", [D, P], f16, kind="ExternalInput").ap()
    wk = nc.dram_tensor("wk", [D, P], f16, kind="ExternalInput").ap()
    wv = nc.dram_tensor("wv", [D, P], f16, kind="ExternalInput").ap()
    wo = nc.dram_tensor("wo", [P, D], f16, kind="ExternalInput").ap()
    bqd = nc.dram_tensor("bq", [P, 1], f32, kind="ExternalInput").ap()
    bkd = nc.dram_tensor("bk", [P, 1], f32, kind="ExternalInput").ap()
    bvd = nc.dram_tensor("bv", [P, 1], f32, kind="ExternalInput").ap()
    out = nc.dram_tensor("out", [BT, D], f16, kind="ExternalOutput").ap()
    dbg = {}
    if debug:
        dbg["qt"] = nc.dram_tensor("dbg_qt", [P, T], f16, kind="ExternalOutput").ap()
        dbg["kt"] = nc.dram_tensor("dbg_kt", [P, T], f16, kind="ExternalOutput").ap()
        dbg["va"] = nc.dram_tensor("dbg_va", [P, KT * 65], f16, kind="ExternalOutput").ap()
        dbg["vb"] = nc.dram_tensor("dbg_vb", [P, KT * 65], f16, kind="ExternalOutput").ap()
        dbg["esc"] = nc.dram_tensor("dbg_esc", [P, 1024], f16, kind="ExternalOutput").ap()
        dbg["ua"] = nc.dram_tensor("dbg_ua", [65, 512], f32, kind="ExternalOutput").ap()
        dbg["ub"] = nc.dram_tensor("dbg_ub", [65, 512], f32, kind="ExternalOutput").ap()
        dbg["rr"] = nc.dram_tensor("dbg_rr", [1, 1024], f16, kind="ExternalOutput").ap()
        dbg["ctq"] = nc.dram_tensor("dbg_ctq", [P, 512], f16, kind="ExternalOutput").ap()

    with tile.TileContext(nc) as tc:
        with (
            tc.tile_pool(name="const", bufs=1) as constp,
            tc.tile_pool(name="xtp", bufs=4) as xtp,
            tc.tile_pool(name="qkv", bufs=1) as qkvp,
            tc.tile_pool(name="vts", bufs=2) as vtsp,
            tc.tile_pool(name="esc", bufs=3) as escp,
            tc.tile_pool(name="ctq", bufs=2) as ctqp,
            tc.tile_pool(name="small", bufs=2) as smallp,
            tc.tile_pool(name="bsb", bufs=2) as bsbp,
            tc.tile_pool(name="posb", bufs=3) as posbp,
            # PSUM: sc 2x2 banks + cx 2x1 banks + flex 2x1 banks = 8
            tc.tile_pool(name="psS", bufs=2, space="PSUM") as psS,
            tc.tile_pool(name="psC", bufs=2, space="PSUM") as psC,
            tc.tile_pool(name="psF", bufs=2, space="PSUM") as psF,
        ):
            # ---- constants / weights; DMA order puts wk and x(0) first so
            # the first projection can start as early as possible ----
            wq_sb = constp.tile([P, DC, P], f16, tag="wq")
            wk_sb = constp.tile([P, DC, P], f16, tag="wk")
            wv_sb = constp.tile([P, DC, P], f16, tag="wv")
            wo_sb = constp.tile([P, D], f16, tag="wo")
            bq_sb = constp.tile([P, 1], f32, tag="bq")
            bk_sb = constp.tile([P, 1], f32, tag="bk")
            bv_sb = constp.tile([P, 1], f32, tag="bv")
            ident_f = constp.tile([P, P], f32, tag="identf")
            make_identity(nc, ident_f)
            ident = constp.tile([P, P], f16, tag="ident")
            nc.vector.tensor_copy(ident, ident_f)
            ones16 = constp.tile([P, 512], f16, tag="ones16")
            nc.vector.memset(ones16, 1.0)

            xt_r = xt.rearrange("(c p) t -> p c t", p=P)
            xtiles = {}

            def load_x(tch):
                def th():
                    t0 = xtp.tile([P, DC, 512], f16, tag="xt", name=f"x{tch}")
                    nc.sync.dma_start(t0, xt_r[:, :, ds(tch * 512, 512)])
                    xtiles[tch] = t0
                return th

            nc.sync.dma_start(wk_sb, wk.rearrange("(c p) e -> p c e", p=P))
            load_x(0)()
            nc.sync.dma_start(bk_sb, bkd)
            nc.sync.dma_start(bq_sb, bqd)
            nc.sync.dma_start(bv_sb, bvd)
            nc.sync.dma_start(wq_sb, wq.rearrange("(c p) e -> p c e", p=P))
            load_x(1)()
            nc.sync.dma_start(wv_sb, wv.rearrange("(c p) e -> p c e", p=P))
            load_x(2)()
            load_x(3)()
            nc.sync.dma_start(wo_sb, wo)

            # ---- per-batch persistent tiles ----
            qt_sb = [
                qkvp.tile([P, T], f16, tag=f"qt{b}", name=f"qt{b}")
                for b in range(NB)
            ]
            kt_sb = [
                qkvp.tile([P, T], f16, tag=f"kt{b}", name=f"kt{b}")
                for b in range(NB)
            ]
            # V natural per batch, 65-wide blocks per ktile: [V_a(64)|1] and
            # [V_b(64)|1]; the ones column accumulates the softmax denominator
            va_sb = [
                qkvp.tile([P, KT * 65], f16, tag=f"va{b}", name=f"va{b}")
                for b in range(NB)
            ]
            vb_sb = [
                qkvp.tile([P, KT * 65], f16, tag=f"vb{b}", name=f"vb{b}")
                for b in range(NB)
            ]
            ones_col = ones16[:, 0:KT].rearrange("p (k one) -> p k one", one=1)
            for b in range(NB):
                nc.vector.tensor_copy(
                    va_sb[b].rearrange("p (k c) -> p k c", c=65)[:, :, 64:65],
                    ones_col,
                )
                nc.vector.tensor_copy(
                    vb_sb[b].rearrange("p (k c) -> p k c", c=65)[:, :, 64:65],
                    ones_col,
                )

            # ---- HAM warmup: keep the PE busy while the first x tile and
            # weights stream in, so K0 runs at 2.4 GHz instead of 1.2 ----
            for w in range(5):
                wt = psS.tile([P, 1024], f32, tag="sc", name="warm")
                nc.tensor.matmul(wt[:, 0:512], ident, ones16,
                                 start=True, stop=True)
                nc.tensor.matmul(wt[:, 512:1024], ident, ones16,
                                 start=True, stop=True)

            _proj_ps = {}

            def proj_half(tch, w_sb, b_sb, dst, half):
                # half 0: open PSUM accumulation, 4 contraction chunks;
                # half 1: 4 more chunks, close group, evict (bias+cast f16)
                def th():
                    if half == 0:
                        ps = psF.tile([P, 512], f32, tag="fx", name="pj")
                        _proj_ps[(tch, id(w_sb))] = ps
                        for c in range(4):
                            nc.tensor.matmul(
                                ps, w_sb[:, c], xtiles[tch][:, c],
                                start=(c == 0), stop=False,
                            )
                    else:
                        ps = _proj_ps.pop((tch, id(w_sb)))
                        for c in range(4, DC):
                            nc.tensor.matmul(
                                ps, w_sb[:, c], xtiles[tch][:, c],
                                start=False, stop=(c == DC - 1),
                            )
                        nc.vector.tensor_scalar_add(dst, ps, b_sb)
                return th

            def K(tch, half):
                b = tch // 4
                dst = kt_sb[b][:, ds((tch % 4) * 512, 512)]
                return proj_half(tch, wk_sb, bk_sb, dst, half)

            def Q(tch, half):
                b = tch // 4
                dst = qt_sb[b][:, ds((tch % 4) * 512, 512)]
                return proj_half(tch, wq_sb, bq_sb, dst, half)

            _vts = {}

            def V(tch, half):
                def th():
                    if half == 0:
                        _vts[tch] = vtsp.tile([P, 512], f32, tag="vts",
                                              name=f"v{tch}")
                    proj_half(tch, wv_sb, bv_sb, _vts[tch], half)()
                return th

            def Vt(tch, half):
                # transpose VT -> V natural; 2 t-tiles of 128 per half.
                # Both transposes into one PSUM tile back-to-back, then the
                # DVE copies drain it (keeps the PE stream dense).
                def th():
                    b = tch // 4
                    vts = _vts[tch]
                    tts = (0, 1) if half == 0 else (2, 3)
                    pvt = psF.tile([P, 256], f32, tag="fx", name="pvt")
                    for j, tt in enumerate(tts):
                        nc.tensor.transpose(
                            pvt[:, ds(j * P, P)], vts[:, ds(tt * P, P)],
                            ident_f,
                        )
                    for j, tt in enumerate(tts):
                        ktile = (tch % 4) * 4 + tt
                        nc.vector.tensor_copy(
                            va_sb[b][:, ds(ktile * 65, DK)],
                            pvt[:, ds(j * P, DK)],
                        )
                        nc.vector.tensor_copy(
                            vb_sb[b][:, ds(ktile * 65, DK)],
                            pvt[:, ds(j * P + DK, DK)],
                        )
                return th

            # ---- the 8 attention chunks as one flat 128-slot pipeline ----
            CHUNKS = [(b, qch) for b in range(NB) for qch in range(4)]
            cstate = [dict() for _ in CHUNKS]

            def fin_thunks(ci):
                # finalize chunk ci: lazy thunks reading cstate[ci], which is
                # populated at the chunk boundary (ua/ub/den0 staged there).
                st = cstate[ci]
                b, qch = CHUNKS[ci]

                def R():
                    # den0[0, 0:512|512:1024] = softmax denominators of both
                    # heads, DMA-staged to partition 0 at the boundary
                    rf = smallp.tile([P, 1024], f32, tag="rf", name="rf")
                    nc.vector.reciprocal_approx_fast(
                        out=rf[0:1, :], in_=st["den0"][0:1, :])
                    rr = smallp.tile([P, 1024], f16, tag="rr", name="rr")
                    nc.vector.tensor_copy(rr[0:1, :], rf[0:1, :])
                    st["rr"] = rr
                    if debug and ci == 0:
                        nc.sync.dma_start(dbg["rr"], rr[0:1, :])

                def bc(which):
                    def th():
                        ps = psF.tile([P, 512], f32, tag="fx", name="bc")
                        nc.tensor.matmul(
                            ps[0:DK, :], ones16[0:1, 0:DK],
                            st["rr"][0:1, ds(which * 512, 512)],
                            start=True, stop=True,
                        )
                        st[f"bc{which}"] = ps
                    return th

                def mul_a():
                    ctq = ctqp.tile([P, 512], f16, tag="ctq", name="ctq")
                    nc.vector.tensor_mul(
                        ctq[0:DK, :], st["ua"][0:DK, :], st["bc0"][0:DK, :])
                    st["ctq"] = ctq

                def mul_b():
                    tmpb = bsbp.tile([DK, 512], f16, tag="tmpb", name="tmpb")
                    nc.vector.tensor_mul(
                        tmpb, st["ub"][0:DK, :], st["bc1"][0:DK, :])
                    nc.sync.dma_start(st["ctq"][DK:P, :], tmpb)

                def op(tt):
                    def th():
                        ctq = st["ctq"]
                        if debug and ci == 0 and tt == 0:
                            nc.sync.dma_start(dbg["ctq"], ctq)
                        po_sb = posbp.tile([P, 1024], f16, tag="po",
                                           name="po_sb")
                        pos = []
                        for half in range(2):
                            po = psF.tile([P, 512], f32, tag="fx", name="po")
                            nc.tensor.matmul(
                                po, ctq[:, ds(tt * P, P)],
                                wo_sb[:, ds(half * 512, 512)],
                                start=True, stop=True,
                            )
                            pos.append(po)
                        for half in range(2):
                            nc.vector.tensor_copy(
                                po_sb[:, ds(half * 512, 512)], pos[half])
                        r0 = b * T + qch * 512 + tt * P
                        nc.sync.dma_start(out[r0: r0 + P, :], po_sb)
                    return th

                return [R, bc(0), bc(1), mul_a, mul_b,
                        op(0), op(1), op(2), op(3)]

            def ctx_mm(ci, kt):
                b, qch = CHUNKS[ci]
                st = cstate[ci]
                e = st["escs"].pop(kt)
                nc.tensor.matmul(
                    st["cxa"], va_sb[b][:, ds(kt * 65, 65)], e[:, 0:512],
                    start=(kt == 0), stop=(kt == KT - 1),
                )
                nc.tensor.matmul(
                    st["cxb"], vb_sb[b][:, ds(kt * 65, 65)], e[:, 512:1024],
                    start=(kt == 0), stop=(kt == KT - 1),
                )

            def boundary(ci):
                # close chunk ci: last ctx, evict accumulators, stage the
                # denominator rows (partition 64) to partition 0 via DMA
                st = cstate[ci]
                ctx_mm(ci, KT - 1)
                ua = bsbp.tile([65, 512], f32, tag="ua", name="ua")
                nc.vector.tensor_copy(ua, st["cxa"])
                ub = bsbp.tile([65, 512], f32, tag="ub", name="ub")
                nc.vector.tensor_copy(ub, st["cxb"])
                st["ua"], st["ub"] = ua, ub
                den0 = smallp.tile([1, 1024], f32, tag="den0", name="den0")
                nc.sync.dma_start(den0[0:1, 0:512], ua[64:65, :])
                nc.sync.dma_start(den0[0:1, 512:1024], ub[64:65, :])
                st["den0"] = den0
                if debug and ci == 0:
                    nc.sync.dma_start(dbg["ua"], ua)
                    nc.sync.dma_start(dbg["ub"], ub)

            def run_chunk(ci, drains):
                b, qch = CHUNKS[ci]
                st = cstate[ci]
                q0 = qch * 512
                st["cxa"] = psC.tile([65, 512], f32, tag="cx", name="cxa")
                st["cxb"] = psC.tile([65, 512], f32, tag="cx", name="cxb")
                st["escs"] = {}
                for kt in range(KT):
                    sc = psS.tile([P, 1024], f32, tag="sc", name="sc")
                    nc.tensor.matmul(
                        sc[:, 0:512],
                        kt_sb[b][0:DK, ds(kt * P, P)],
                        qt_sb[b][0:DK, ds(q0, 512)],
                        start=True, stop=True,
                    )
                    nc.tensor.matmul(
                        sc[:, 512:1024],
                        kt_sb[b][DK:P, ds(kt * P, P)],
                        qt_sb[b][DK:P, ds(q0, 512)],
                        start=True, stop=True,
                        tile_position=(64, 0),
                    )
                    esc = escp.tile([P, 1024], f16, tag="esc", name="esc")
                    nc.scalar.activation(esc, sc, Exp, scale=0.125)
                    st["escs"][kt] = esc
                    if debug and ci == 0 and kt == 0:
                        nc.sync.dma_start(dbg["esc"], esc)
                    if kt > 0:
                        ctx_mm(ci, kt - 1)
                    elif ci > 0:
                        boundary(ci - 1)
                    for th in drains[kt]:
                        th()

            def sched(*slots):
                d = [[] for _ in range(KT)]
                for i, s in enumerate(slots):
                    if s:
                        d[i] = list(s) if isinstance(s, (list, tuple)) else [s]
                return d

            # head: project K0/Q0/V0, transpose V0
            K(0, 0)(); K(0, 1)()
            Q(0, 0)(); Q(0, 1)()
            V(0, 0)(); V(0, 1)()
            Vt(0, 0)(); Vt(0, 1)()

            # chunk 0: remaining b0 projections
            run_chunk(0, sched(
                [K(1, 0), K(1, 1)], [V(1, 0), V(1, 1)], [Vt(1, 0), Vt(1, 1)],
                [K(2, 0), K(2, 1)], [V(2, 0), V(2, 1)], [Vt(2, 0), Vt(2, 1)],
                K(3, 0), K(3, 1), V(3, 0), V(3, 1), Vt(3, 0), Vt(3, 1),
                Q(1, 0), Q(1, 1), None, None,
            ))
            f = fin_thunks(0)
            run_chunk(1, sched(
                [load_x(4), f[0]], f[1], f[2], f[3], f[4], Q(2, 0), Q(2, 1),
                f[5], f[6], f[7], f[8], K(4, 0), K(4, 1),
                [load_x(5), V(4, 0)], V(4, 1), [Vt(4, 0), Vt(4, 1)],
            ))
            f = fin_thunks(1)
            run_chunk(2, sched(
                [Q(3, 0), f[0]], Q(3, 1), f[1], f[2], f[3], f[4],
                Q(4, 0), Q(4, 1), f[5], f[6], f[7], f[8],
                [load_x(6), K(5, 0)], K(5, 1), [V(5, 0), V(5, 1)],
                [Vt(5, 0), Vt(5, 1)],
            ))
            f = fin_thunks(2)
            run_chunk(3, sched(
                f[0], f[1], f[2], f[3], f[4], f[5], f[6], f[7], f[8],
                [load_x(7), K(6, 0)], K(6, 1), V(6, 0), V(6, 1),
                [Vt(6, 0), Vt(6, 1)], None, None,
            ))
            f = fin_thunks(3)
            run_chunk(4, sched(
                [K(7, 0), f[0]], K(7, 1), V(7, 0), V(7, 1), f[1], f[2],
                f[3], f[4], [Vt(7, 0), Vt(7, 1)], Q(5, 0), Q(5, 1),
                f[5], f[6], f[7], f[8], None,
            ))
            f = fin_thunks(4)
            run_chunk(5, sched(
                f[0], f[1], f[2], f[3], f[4], Q(6, 0), Q(6, 1),
                f[5], f[6], f[7], f[8], None, None, None, None, None,
            ))
            f = fin_thunks(5)
            run_chunk(6, sched(
                f[0], f[1], f[2], f[3], f[4], Q(7, 0), Q(7, 1),
                f[5], f[6], f[7], f[8], None, None, None, None, None,
            ))
            f = fin_thunks(6)
            run_chunk(7, sched(
                f[0], f[1], f[2], f[3], f[4], f[5], f[6], f[7], f[8],
                None, None, None, None, None, None, None,
            ))
            # tail: close and finalize the last chunk
            boundary(7)
            for th in fin_thunks(7):
                th()
            if debug:
                nc.sync.dma_start(dbg["qt"], qt_sb[0])
                nc.sync.dma_start(dbg["kt"], kt_sb[0])
                nc.sync.dma_start(dbg["va"], va_sb[0])
                nc.sync.dma_start(dbg["vb"], vb_sb[0])

    nc.compile()
    return nc


def _get_nc(reps=1, debug=False):
    key = f"nc{reps}_{debug}"
    if key not in _CACHE:
        _CACHE[key] = _build(reps, debug=debug)
    return _CACHE[key]


def kernel(x, Wq, bq, Wk, bk, Wv, bv, Wo, bo):
    from concourse.bass_utils import run_bass_kernel_spmd

    x = np.asarray(x, dtype=np.float32)
    Wq = np.asarray(Wq, dtype=np.float32)
    Wk = np.asarray(Wk, dtype=np.float32)
    Wv = np.asarray(Wv, dtype=np.float32)
    Wo = np.asarray(Wo, dtype=np.float32)
    bq = np.asarray(bq, dtype=np.float32)
    bk = np.asarray(bk, dtype=np.float32)
    bv = np.asarray(bv, dtype=np.float32)
    bo = np.asarray(bo, dtype=np.float32)

    B, Tl, Dl = x.shape
    xt = np.ascontiguousarray(x.reshape(B * Tl, Dl).T.astype(np.float16))

    in_maps = []
    for c in range(NCORES):
        sl = slice(c * P, (c + 1) * P)
        in_maps.append(
            {
                "xt": xt,
                "wq": np.ascontiguousarray(Wq[sl, :].T.astype(np.float16)),
                "wk": np.ascontiguousarray(Wk[sl, :].T.astype(np.float16)),
                "wv": np.ascontiguousarray(Wv[sl, :].T.astype(np.float16)),
                "wo": np.ascontiguousarray(Wo[:, sl].T.astype(np.float16)),
                "bq": np.ascontiguousarray(bq[sl].reshape(P, 1)),
                "bk": np.ascontiguousarray(bk[sl].reshape(P, 1)),
                "bv": np.ascontiguousarray(bv[sl].reshape(P, 1)),
            }
        )

    nc = _get_nc()
    _CACHE["in_maps"] = in_maps
    res = run_bass_kernel_spmd(nc, in_maps, core_ids=list(range(NCORES)))
    acc = res.results[0]["out"].astype(np.float32)
    for c in range(1, NCORES):
        acc = acc + res.results[c]["out"].astype(np.float32)
    acc = acc + bo[None, :]
    return acc.reshape(B, Tl, Dl).astype(np.float32)
